# revision 38
# baseline (speedup 1.0000x reference)
"""BaseAttentionPooling Trainium2 kernel.

reference:
    h = tanh(x @ W1 + b1)            # [N, H]
    logits = (h @ W2 + b2)[:, 0]     # [N]
    per-graph softmax over sorted `batch`, pooled = seg_sum(x * w)  # [G, D]

Strategy (data-parallel over graphs, 8 cores, SPMD-identical program):
  - 512 graphs/core, 8 blocks of 64 graphs; per-block chunk counts
    (max over cores) keep the program SPMD-uniform with ~1.5% padding.
  - Everything ships in fp8 e3m4 (4 mantissa bits): a node-major copy
    with a ones column (pooling numerator + denominator in one 257-col
    matmul) and a d-major two-k-tile copy (MLP input). W1 is scaled by
    16 into e3m4's normal range; the tanh activation rescales by 1/16.
    e3m4 beats e4m3 ~2x on both logits and pooling error; host-side
    simulation of the exact dtype chain matches hardware to 1e-6.
  - Rows are merged [xs | xt] and laid out partition-major so one DMA
    covers 2 groups with a single contiguous descriptor per partition.
  - b2 dropped (cancels in softmax); exp without max-subtraction is safe
    because |logits| <= ||W2||_1 + |b2| is small.
  - PE per 8-chunk group: 4x 512-col k-tile matmuls (hT, e3m4), 8x
    1-col stationary-hsb matmuls (logits), 8x 257-col stationary-oh
    matmuls (pooled+den, bf16 stat x e3m4 moving). Matmul cost is
    1 moving column/cycle regardless of dtype, so fp8 wins on DMA
    bytes; LDWEIGHTS hides under matmuls via the PE reorder window.
  - oh[i, g] = (iota[g] == rel_gid[i]) * e[i]: indicator built once per
    group on DVE (broadcast APs, off the critical path); exp + oh-mult
    split into two half-group waves so the first wave is gated only by
    the first tanh; pooled flush deferred two groups.
  - PSUM: 4 block-pair accumulators [128, 257] + 2 hT + 2 logit banks.
"""

import os
import sys

import numpy as np

for _p in ("/opt/trn_rl_repo",):
    if _p not in sys.path and os.path.isdir(_p):
        sys.path.insert(0, _p)

import ml_dtypes

import concourse.bass as bass
import concourse.tile as tile
from concourse import bacc, mybir
from concourse import bass_utils

N, D, H, G = 500000, 256, 128, 4096
NCORES = 8
GPC = G // NCORES          # graphs per core = 512
BLKG = 64                  # graphs per block
NBLK = GPC // BLKG         # blocks per core = 8
P = 128                    # partition / chunk size

BF16 = mybir.dt.bfloat16
F32 = mybir.dt.float32
F8E3 = mybir.dt.float8e3
NP_BF16 = ml_dtypes.bfloat16
NP_E3 = ml_dtypes.float8_e3m4
W1SCALE = 16.0   # lift W1 (|W1|<=1/16) out of e3m4's denormal range

LAST_RESULT = None  # test.py reads exec_time_ns / profile from here


# ---------------------------------------------------------------- host plan

def make_plan(batch):
    """Per-block chunk counts (max across cores, SPMD-uniform program)."""
    batch = np.asarray(batch)
    seg = np.searchsorted(batch, np.arange(G + 1), side="left")  # [G+1]
    counts = np.zeros((NCORES, NBLK), dtype=np.int64)
    for c in range(NCORES):
        for b in range(NBLK):
            g0 = c * GPC + b * BLKG
            counts[c, b] = seg[g0 + BLKG] - seg[g0]
    cpbs = np.ceil(counts.max(axis=0) / P).astype(np.int64)  # [NBLK]
    ch = int(cpbs.sum())
    ch8 = ((ch + 15) // 16) * 16   # whole 8-chunk groups, even group count
    cpbs[-1] += ch8 - ch
    starts = np.concatenate([[0], np.cumsum(cpbs)])  # [NBLK+1]
    return seg, cpbs, starts, ch8


def build_inputs(x, batch, W1, b1, W2, seg, starts, ch):
    """Build the 8 per-core input maps (layout/precision prep only)."""
    x = np.asarray(x)
    batch = np.asarray(batch)
    n_g8 = ch // 8

    w1_f = np.asarray(W1, dtype=np.float32) * W1SCALE  # [256, 128] scaled
    # two k-tile stationaries, e3m4 (tanh activation divides by W1SCALE)
    w1a = np.ascontiguousarray(w1_f[0:P, :]).astype(NP_E3)
    w1b = np.ascontiguousarray(w1_f[P : 2 * P, :]).astype(NP_E3)
    b1_f = np.asarray(b1, dtype=np.float32).reshape(H, 1)
    w2_bf = np.asarray(W2, dtype=np.float32).astype(NP_BF16).reshape(H, 1)
    # io8[p, j*64+g] = g  (iota tiled per chunk-of-group)
    io8 = np.broadcast_to(
        np.arange(BLKG, dtype=np.float32), (P, 8, BLKG)
    ).reshape(P, 8 * BLKG).astype(NP_BF16)

    in_maps = []
    for c in range(NCORES):
        xpad = np.zeros((ch * P, D), dtype=np.float32)
        rel = np.full(ch * P, -1.0, dtype=np.float32)
        for b in range(NBLK):
            g0 = c * GPC + b * BLKG
            s0, s1 = int(seg[g0]), int(seg[g0 + BLKG])
            n = s1 - s0
            r0 = int(starts[b]) * P
            xpad[r0 : r0 + n] = x[s0:s1]
            rel[r0 : r0 + n] = (batch[s0:s1] - g0).astype(np.float32)
        # node-major fp8 e3m4 with a ones-column appended per chunk (row
        # [x(256) | 1] so one 257-col matmul accumulates pooled AND den),
        # tiled so each 8-chunk group is one contiguous DRAM block
        xaug = np.ones((ch * P, D + 1), dtype=NP_E3)
        xaug[:, :D] = xpad.astype(NP_E3)
        xs_t = np.ascontiguousarray(
            xaug.reshape(n_g8, 8, P, D + 1)
            .transpose(0, 2, 1, 3)
            .reshape(n_g8 * P, 8 * (D + 1))
        )
        # d-major fp8 e3m4, two k-tile blocks per group:
        # xt_t[g8*128+p, t*1024 + j*128 + i] = xpad[(g8*8+j)*128+i, t*128+p]
        xt_t = np.ascontiguousarray(
            xpad.astype(NP_E3)
            .reshape(n_g8, 8, P, 2, P)   # [g8, j, i, t, p]
            .transpose(0, 4, 3, 1, 2)    # [g8, p, t, j, i]
            .reshape(n_g8 * P, 2 * 8 * P)
        )
        blr = np.ascontiguousarray(
            rel.reshape(ch, P).T.astype(NP_BF16)
        )  # [128, ch] bf16 (rel ids <= 63 exact)
        # merge: row = [xs_row (8*257 e3m4) | xt_row (2*8*128 e3m4)], then
        # partition-major [p, g8, row] so one DMA can cover several groups
        # with a single long contiguous descriptor per partition
        xm = np.concatenate(
            [xs_t.view(np.uint8), xt_t.view(np.uint8)], axis=1
        )
        rowb = xm.shape[1]
        xm = np.ascontiguousarray(
            xm.reshape(n_g8, P, rowb).transpose(1, 0, 2).reshape(P, n_g8 * rowb)
        )
        in_maps.append(
            {
                "xm": xm,
                "blr": blr,
                "w1a": w1a,
                "w1b": w1b,
                "b1": b1_f,
                "w2": w2_bf,
                "io8": io8,
            }
        )
    return in_maps


# ------------------------------------------------------------- bass program

def build_bass(ch, starts):
    """Build the SPMD-uniform per-core program."""
    nc = bacc.Bacc(
        "TRN2",
        target_bir_lowering=False,
        debug=False,
        num_devices=NCORES,
    )
    n_g8 = ch // 8
    ROWB = 8 * (D + 1) + 2 * 8 * P   # merged row bytes (all fp8)
    XSB = 8 * (D + 1)
    xm = nc.dram_tensor("xm", [P, n_g8 * ROWB], mybir.dt.uint8,
                        kind="ExternalInput").ap()
    blr = nc.dram_tensor("blr", [P, ch], BF16, kind="ExternalInput").ap()
    w1a_d = nc.dram_tensor("w1a", [P, H], F8E3, kind="ExternalInput").ap()
    w1b_d = nc.dram_tensor("w1b", [P, H], F8E3, kind="ExternalInput").ap()
    b1 = nc.dram_tensor("b1", [H, 1], F32, kind="ExternalInput").ap()
    w2 = nc.dram_tensor("w2", [H, 1], BF16, kind="ExternalInput").ap()
    io8 = nc.dram_tensor("io8", [P, 8 * BLKG], BF16, kind="ExternalInput").ap()
    out = nc.dram_tensor("out", [GPC, D], F32, kind="ExternalOutput").ap()

    with tile.TileContext(nc) as tc:
        with (
            tc.tile_pool(name="consts", bufs=1) as cpool,
            tc.tile_pool(name="xb", bufs=6) as xbpool,
            tc.tile_pool(name="hsb", bufs=4) as hsbpool,
            tc.tile_pool(name="e8", bufs=3) as epool,
            tc.tile_pool(name="oh", bufs=4) as ohpool,
            tc.tile_pool(name="outsb", bufs=2) as outpool,
            tc.tile_pool(name="acc", bufs=1, space="PSUM") as accpool,
            tc.tile_pool(name="hps", bufs=2, space="PSUM") as hpool,
            tc.tile_pool(name="lg", bufs=2, space="PSUM") as lgpool,
        ):
            # ---- constants into SBUF
            w1a_sb = cpool.tile([P, H], F8E3, tag="w1a")
            w1b_sb = cpool.tile([P, H], F8E3, tag="w1b")
            b1_sb = cpool.tile([H, 1], F32, tag="b1")
            w2_sb = cpool.tile([H, 1], BF16, tag="w2")
            io_sb = cpool.tile([P, 8 * BLKG], BF16, tag="io8")
            blr_sb = cpool.tile([P, ch], BF16, tag="blr")
            nc.scalar.dma_start(w1a_sb[:], w1a_d[:])
            nc.scalar.dma_start(w1b_sb[:], w1b_d[:])
            nc.scalar.dma_start(b1_sb[:], b1[:])
            nc.scalar.dma_start(w2_sb[:], w2[:])
            nc.scalar.dma_start(io_sb[:], io8[:])
            nc.scalar.dma_start(blr_sb[:], blr[:])
            # ---- persistent accumulators (PSUM)
            # col 0..255 pooled, col 256 denominator; two 64-graph blocks
            # pack into each [128, 257] f32 tile via partition halves
            pp = [
                accpool.tile([P, D + 1], F32, tag=f"pp{t}", name=f"pp{t}")
                for t in range(4)
            ]

            def pooled_out(b):
                r0 = (b % 2) * BLKG
                return pp[b // 2][r0 : r0 + BLKG, :]

            def blk(c):
                return int(np.searchsorted(starts, c, side="right")) - 1

            def flush_one(item, lg_thunk=None):
                # pooled[g, 0:256] += oh.T @ x ; den = col 256 (x ones col).
                oh_ap, xb, j, c = item
                b = blk(c)
                first = c == int(starts[b])
                last = c == int(starts[b + 1]) - 1
                W = D + 1
                nc.tensor.matmul(
                    pooled_out(b),
                    oh_ap,
                    xb[:, j * W : (j + 1) * W],
                    start=first,
                    stop=last,
                )
                if lg_thunk is not None:
                    lg_thunk()

            # software pipeline: per iteration the PE stream is
            #   [DR(g), pool-flush(g-1) x8, logits(g) x8]
            # so the pooled burst of the previous group covers the tanh
            # latency of this one; is_eq (constants only) is hoisted off
            # the exp critical path.
            xmt = None
            prevq = []           # deferred (oh8a, oh8b, xb, c0) groups
            for g8 in range(n_g8):
                if g8 % 2 == 0:
                    xmt = xbpool.tile([P, 2 * ROWB], mybir.dt.uint8)
                    nc.sync.dma_start(
                        xmt[:], xm[:, g8 * ROWB : (g8 + 2) * ROWB]
                    )
                base = (g8 % 2) * ROWB
                xb = xmt[:, base : base + XSB].bitcast(F8E3)  # [128, 8*257]
                xt_ap = (
                    xmt[:, base + XSB : base + ROWB]
                    .bitcast(F8E3)
                    .rearrange("p (t n) -> p t n", t=2)
                )  # [128, 2, 8*128]
                lg = lgpool.tile([P, 8], F32)
                hsbs = []
                # hT: two e3m4 k-tile matmuls per 4-chunk batch; tanh
                # rescales by 1/W1SCALE
                for j0 in range(0, 8, 4):
                    hps = hpool.tile([P, 4 * P], F32)
                    nc.tensor.matmul(
                        hps[:],
                        w1a_sb[:],
                        xt_ap[:, 0, j0 * P : (j0 + 4) * P],
                        start=True,
                        stop=False,
                    )
                    nc.tensor.matmul(
                        hps[:],
                        w1b_sb[:],
                        xt_ap[:, 1, j0 * P : (j0 + 4) * P],
                        start=False,
                        stop=True,
                    )
                    hsb = hsbpool.tile([P, 4 * P], BF16)
                    nc.scalar.activation(
                        hsb[:], hps[:],
                        mybir.ActivationFunctionType.Tanh, bias=b1_sb[:],
                        scale=1.0 / W1SCALE,
                    )
                    hsbs.append(hsb)
                # indicator for this group: depends only on constants
                c0 = g8 * 8
                ind8 = ohpool.tile([P, 8 * BLKG], BF16, tag="ind8")
                io_v = io_sb[:].rearrange("p (j g) -> p j g", j=8)
                blr_v = blr_sb[:, c0 : c0 + 8].unsqueeze(-1).broadcast_to(
                    [P, 8, BLKG]
                )
                ind8_v = ind8[:].rearrange("p (j g) -> p j g", j=8)
                nc.vector.tensor_tensor(
                    ind8_v, io_v, blr_v, mybir.AluOpType.is_equal
                )
                # previous group's pooled flush (oh8/xb ready) keeps the
                # PE busy while this group's tanh runs on Scalar
                if len(prevq) >= 2:
                    poa, pob, pxb, pc0 = prevq.pop(0)
                    for j in range(8):
                        po = (poa, pob)[j // 4]
                        flush_one(
                            (po[:, (j % 4) * BLKG : (j % 4 + 1) * BLKG],
                             pxb, j, pc0 + j)
                        )
                # logits, exp and oh in two half-group waves so the oh
                # tail latency halves: wave a gated only by tanh0; the two
                # oh multiplies run concurrently on DVE and GpSimd
                oh8a = ohpool.tile([P, 4 * BLKG], BF16, tag="oh8a")
                oh8b = ohpool.tile([P, 4 * BLKG], BF16, tag="oh8b")
                for h in range(2):
                    for j in range(4 * h, 4 * h + 4):
                        nc.tensor.matmul(
                            lg[:, j : j + 1],
                            hsbs[j // 4][:, (j % 4) * P : (j % 4 + 1) * P],
                            w2_sb[:],
                            start=True,
                            stop=True,
                        )
                    e4 = epool.tile([P, 4], BF16, tag=f"e4{h}")
                    nc.scalar.activation(
                        e4[:], lg[:, 4 * h : 4 * h + 4],
                        mybir.ActivationFunctionType.Exp,
                    )
                    ohh = (oh8a, oh8b)[h]
                    e4_v = e4[:].unsqueeze(-1).broadcast_to([P, 4, BLKG])
                    ohh_v = ohh[:].rearrange("p (j g) -> p j g", j=4)
                    nc.vector.tensor_tensor(
                        ohh_v, ind8_v[:, 4 * h : 4 * h + 4, :], e4_v,
                        mybir.AluOpType.mult,
                    )
                prevq.append((oh8a, oh8b, xb, c0))
            for poa, pob, pxb, pc0 in prevq:
                for j in range(8):
                    po = (poa, pob)[j // 4]
                    flush_one(
                        (po[:, (j % 4) * BLKG : (j % 4 + 1) * BLKG],
                         pxb, j, pc0 + j)
                    )

            # ---- epilogue: out[g] = pooled[g] / max(denom[g], tiny)
            recs = []
            for b in range(NBLK):
                acc = pooled_out(b)
                dmax = outpool.tile([BLKG, 1], F32, tag=f"dmax{b}", name=f"dmax{b}")
                rec = outpool.tile([BLKG, 1], F32, tag=f"rec{b}", name=f"rec{b}")
                nc.vector.tensor_scalar_max(dmax[:], acc[:, D : D + 1], 1e-30)
                nc.vector.reciprocal(rec[:], dmax[:])
                recs.append(rec)
            for b in range(NBLK):
                osb = outpool.tile([BLKG, D], F32, tag="osb")
                nc.scalar.mul(osb[:], pooled_out(b)[:, 0:D], recs[b][:])
                nc.sync.dma_start(out[b * BLKG : (b + 1) * BLKG, :], osb[:])

    nc.compile()
    return nc


# ----------------------------------------------------------------- kernel()

def kernel(**inputs):
    global LAST_RESULT
    x = np.asarray(inputs["x"])
    batch = np.asarray(inputs["batch"])
    W1 = np.asarray(inputs["W1"])
    b1 = np.asarray(inputs["b1"])
    W2 = np.asarray(inputs["W2"])
    # b2 cancels in the softmax; unused.

    seg, cpbs, starts, ch = make_plan(batch)
    in_maps = build_inputs(x, batch, W1, b1, W2, seg, starts, ch)
    nc = build_bass(ch, starts)
    res = bass_utils.run_bass_kernel_spmd(
        nc, in_maps, list(range(NCORES))
    )
    LAST_RESULT = res
    out = np.concatenate(
        [np.asarray(res.results[c]["out"]) for c in range(NCORES)], axis=0
    )
    return out.astype(np.float32)


# revision 39
# speedup vs baseline: 1.1586x; 1.1586x over previous
"""BaseAttentionPooling Trainium2 kernel.

reference:
    h = tanh(x @ W1 + b1)            # [N, H]
    logits = (h @ W2 + b2)[:, 0]     # [N]
    per-graph softmax over sorted `batch`, pooled = seg_sum(x * w)  # [G, D]

Strategy (data-parallel over graphs, 8 cores, SPMD-identical program):
  - 512 graphs/core, 8 blocks of 64 graphs; per-block chunk counts
    (max over cores) keep the program SPMD-uniform with ~1.5% padding.
  - Everything ships in fp8 e3m4 (4 mantissa bits): a node-major copy
    with a ones column (pooling numerator + denominator in one 257-col
    matmul) and a d-major two-k-tile copy (MLP input). W1 is scaled by
    16 into e3m4's normal range; the tanh activation rescales by 1/16.
    e3m4 beats e4m3 ~2x on both logits and pooling error; host-side
    simulation of the exact dtype chain matches hardware to 1e-6.
  - Rows are merged [xs | xt] and laid out partition-major so one DMA
    covers 2 groups with a single contiguous descriptor per partition.
  - b2 dropped (cancels in softmax); exp without max-subtraction is safe
    because |logits| <= ||W2||_1 + |b2| is small.
  - PE per 8-chunk group: 4x 512-col k-tile matmuls (hT, e3m4), 8x
    1-col stationary-hsb matmuls (logits), 8x 257-col stationary-oh
    matmuls (pooled+den, bf16 stat x e3m4 moving). Matmul cost is
    1 moving column/cycle regardless of dtype, so fp8 wins on DMA
    bytes; LDWEIGHTS hides under matmuls via the PE reorder window.
  - oh[i, g] = (iota[g] == rel_gid[i]) * e[i]: indicator built once per
    group on DVE (broadcast APs, off the critical path); exp + oh-mult
    split into two half-group waves so the first wave is gated only by
    the first tanh; pooled flush deferred two groups.
  - PSUM: 4 block-pair accumulators [128, 257] + 2 hT + 2 logit banks.
"""

import os
import sys

import numpy as np

for _p in ("/opt/trn_rl_repo",):
    if _p not in sys.path and os.path.isdir(_p):
        sys.path.insert(0, _p)

import ml_dtypes

import concourse.bass as bass
import concourse.tile as tile
from concourse import bacc, mybir
from concourse import bass_utils

N, D, H, G = 500000, 256, 128, 4096
NCORES = 8
GPC = G // NCORES          # graphs per core = 512
BLKG = 64                  # graphs per block
NBLK = GPC // BLKG         # blocks per core = 8
P = 128                    # partition / chunk size

BF16 = mybir.dt.bfloat16
F32 = mybir.dt.float32
F8E3 = mybir.dt.float8e3
NP_BF16 = ml_dtypes.bfloat16
NP_E3 = ml_dtypes.float8_e3m4
W1SCALE = 16.0   # lift W1 (|W1|<=1/16) out of e3m4's denormal range

LAST_RESULT = None  # test.py reads exec_time_ns / profile from here


# ---------------------------------------------------------------- host plan

def make_plan(batch):
    """Per-block chunk counts (max across cores, SPMD-uniform program)."""
    batch = np.asarray(batch)
    seg = np.searchsorted(batch, np.arange(G + 1), side="left")  # [G+1]
    counts = np.zeros((NCORES, NBLK), dtype=np.int64)
    for c in range(NCORES):
        for b in range(NBLK):
            g0 = c * GPC + b * BLKG
            counts[c, b] = seg[g0 + BLKG] - seg[g0]
    cpbs = np.ceil(counts.max(axis=0) / P).astype(np.int64)  # [NBLK]
    ch = int(cpbs.sum())
    ch8 = ((ch + 15) // 16) * 16   # whole 8-chunk groups, even group count
    cpbs[-1] += ch8 - ch
    starts = np.concatenate([[0], np.cumsum(cpbs)])  # [NBLK+1]
    return seg, cpbs, starts, ch8


def build_inputs(x, batch, W1, b1, W2, seg, starts, ch):
    """Build the 8 per-core input maps (layout/precision prep only)."""
    x = np.asarray(x)
    batch = np.asarray(batch)
    n_g8 = ch // 8

    w1_f = np.asarray(W1, dtype=np.float32) * W1SCALE  # [256, 128] scaled
    # two k-tile stationaries, e3m4 (tanh activation divides by W1SCALE)
    w1a = np.ascontiguousarray(w1_f[0:P, :]).astype(NP_E3)
    w1b = np.ascontiguousarray(w1_f[P : 2 * P, :]).astype(NP_E3)
    b1_f = np.asarray(b1, dtype=np.float32).reshape(H, 1)
    w2_bf = np.asarray(W2, dtype=np.float32).astype(NP_BF16).reshape(H, 1)
    # io8[p, j*64+g] = g  (iota tiled per chunk-of-group)
    io8 = np.broadcast_to(
        np.arange(BLKG, dtype=np.float32), (P, 8, BLKG)
    ).reshape(P, 8 * BLKG).astype(NP_BF16)

    in_maps = []
    for c in range(NCORES):
        xpad = np.zeros((ch * P, D), dtype=np.float32)
        rel = np.full(ch * P, -1.0, dtype=np.float32)
        for b in range(NBLK):
            g0 = c * GPC + b * BLKG
            s0, s1 = int(seg[g0]), int(seg[g0 + BLKG])
            n = s1 - s0
            r0 = int(starts[b]) * P
            xpad[r0 : r0 + n] = x[s0:s1]
            rel[r0 : r0 + n] = (batch[s0:s1] - g0).astype(np.float32)
        # node-major fp8 e3m4 with a ones-column appended per chunk (row
        # [x(256) | 1] so one 257-col matmul accumulates pooled AND den),
        # tiled so each 8-chunk group is one contiguous DRAM block
        xaug = np.ones((ch * P, D + 1), dtype=NP_E3)
        xaug[:, :D] = xpad.astype(NP_E3)
        xs_t = np.ascontiguousarray(
            xaug.reshape(n_g8, 8, P, D + 1)
            .transpose(0, 2, 1, 3)
            .reshape(n_g8 * P, 8 * (D + 1))
        )
        # d-major fp8 e3m4, two k-tile blocks per group:
        # xt_t[g8*128+p, t*1024 + j*128 + i] = xpad[(g8*8+j)*128+i, t*128+p]
        xt_t = np.ascontiguousarray(
            xpad.astype(NP_E3)
            .reshape(n_g8, 8, P, 2, P)   # [g8, j, i, t, p]
            .transpose(0, 4, 3, 1, 2)    # [g8, p, t, j, i]
            .reshape(n_g8 * P, 2 * 8 * P)
        )
        blr = np.ascontiguousarray(
            rel.reshape(ch, P).T.astype(NP_BF16)
        )  # [128, ch] bf16 (rel ids <= 63 exact)
        # merge: row = [xs_row (8*257 e3m4) | xt_row (2*8*128 e3m4)], then
        # partition-major [p, g8, row] so one DMA can cover several groups
        # with a single long contiguous descriptor per partition
        xm = np.concatenate(
            [xs_t.view(np.uint8), xt_t.view(np.uint8)], axis=1
        )
        rowb = xm.shape[1]
        xm = np.ascontiguousarray(
            xm.reshape(n_g8, P, rowb).transpose(1, 0, 2).reshape(P, n_g8 * rowb)
        )
        in_maps.append(
            {
                "xm": xm,
                "blr": blr,
                "w1a": w1a,
                "w1b": w1b,
                "b1": b1_f,
                "w2": w2_bf,
                "io8": io8,
            }
        )
    return in_maps


# ------------------------------------------------------------- bass program

def build_bass(ch, starts):
    """Build the SPMD-uniform per-core program."""
    nc = bacc.Bacc(
        "TRN2",
        target_bir_lowering=False,
        debug=False,
        num_devices=NCORES,
    )
    n_g8 = ch // 8
    ROWB = 8 * (D + 1) + 2 * 8 * P   # merged row bytes (all fp8)
    XSB = 8 * (D + 1)
    xm = nc.dram_tensor("xm", [P, n_g8 * ROWB], mybir.dt.uint8,
                        kind="ExternalInput").ap()
    blr = nc.dram_tensor("blr", [P, ch], BF16, kind="ExternalInput").ap()
    w1a_d = nc.dram_tensor("w1a", [P, H], F8E3, kind="ExternalInput").ap()
    w1b_d = nc.dram_tensor("w1b", [P, H], F8E3, kind="ExternalInput").ap()
    b1 = nc.dram_tensor("b1", [H, 1], F32, kind="ExternalInput").ap()
    w2 = nc.dram_tensor("w2", [H, 1], BF16, kind="ExternalInput").ap()
    io8 = nc.dram_tensor("io8", [P, 8 * BLKG], BF16, kind="ExternalInput").ap()
    out = nc.dram_tensor("out", [GPC, D], F32, kind="ExternalOutput").ap()

    with tile.TileContext(nc) as tc:
        with (
            tc.tile_pool(name="consts", bufs=1) as cpool,
            tc.tile_pool(name="xb", bufs=6) as xbpool,
            tc.tile_pool(name="hsb", bufs=4) as hsbpool,
            tc.tile_pool(name="e8", bufs=4) as epool,
            tc.tile_pool(name="oh", bufs=6) as ohpool,
            tc.tile_pool(name="outsb", bufs=2) as outpool,
            tc.tile_pool(name="acc", bufs=1, space="PSUM") as accpool,
            tc.tile_pool(name="hps", bufs=2, space="PSUM") as hpool,
            tc.tile_pool(name="lg", bufs=2, space="PSUM") as lgpool,
        ):
            # ---- constants into SBUF
            w1a_sb = cpool.tile([P, H], F8E3, tag="w1a")
            w1b_sb = cpool.tile([P, H], F8E3, tag="w1b")
            b1_sb = cpool.tile([H, 1], F32, tag="b1")
            w2_sb = cpool.tile([H, 1], BF16, tag="w2")
            io_sb = cpool.tile([P, 8 * BLKG], BF16, tag="io8")
            blr_sb = cpool.tile([P, ch], BF16, tag="blr")
            nc.scalar.dma_start(w1a_sb[:], w1a_d[:])
            nc.scalar.dma_start(w1b_sb[:], w1b_d[:])
            nc.scalar.dma_start(b1_sb[:], b1[:])
            nc.scalar.dma_start(w2_sb[:], w2[:])
            nc.scalar.dma_start(io_sb[:], io8[:])
            nc.scalar.dma_start(blr_sb[:], blr[:])
            # ---- persistent accumulators (PSUM)
            # col 0..255 pooled, col 256 denominator; two 64-graph blocks
            # pack into each [128, 257] f32 tile via partition halves
            pp = [
                accpool.tile([P, D + 1], F32, tag=f"pp{t}", name=f"pp{t}")
                for t in range(4)
            ]

            def pooled_out(b):
                r0 = (b % 2) * BLKG
                return pp[b // 2][r0 : r0 + BLKG, :]

            def blk(c):
                return int(np.searchsorted(starts, c, side="right")) - 1

            def flush_one(item, lg_thunk=None):
                # pooled[g, 0:256] += oh.T @ x ; den = col 256 (x ones col).
                oh_ap, xb, j, c = item
                b = blk(c)
                first = c == int(starts[b])
                last = c == int(starts[b + 1]) - 1
                W = D + 1
                nc.tensor.matmul(
                    pooled_out(b),
                    oh_ap,
                    xb[:, j * W : (j + 1) * W],
                    start=first,
                    stop=last,
                )
                if lg_thunk is not None:
                    lg_thunk()

            # software pipeline, two groups per phase to minimize
            # moving-operand stream switches (xt -> xs -> xt -> w2):
            #   [MLP(g0), pool-flush(g0-2, g0-1) x16, MLP(g1),
            #    logits(g0) x8, logits(g1) x8]
            # the 16-chunk pooled burst covers both groups' tanh latency.
            prevq = []           # deferred (oh8a, oh8b, xb, c0) groups
            for it in range(n_g8 // 2):
                g0 = 2 * it
                xmt = xbpool.tile([P, 2 * ROWB], mybir.dt.uint8)
                nc.sync.dma_start(
                    xmt[:], xm[:, g0 * ROWB : (g0 + 2) * ROWB]
                )
                gdat = []
                for g8 in (g0, g0 + 1):
                    base = (g8 % 2) * ROWB
                    xb = xmt[:, base : base + XSB].bitcast(F8E3)
                    xt_ap = (
                        xmt[:, base + XSB : base + ROWB]
                        .bitcast(F8E3)
                        .rearrange("p (t n) -> p t n", t=2)
                    )  # [128, 2, 8*128]
                    gdat.append((g8, xb, xt_ap))

                def mlp(g8, xt_ap):
                    hsbs = []
                    for j0 in range(0, 8, 4):
                        hps = hpool.tile([P, 4 * P], F32)
                        nc.tensor.matmul(
                            hps[:],
                            w1a_sb[:],
                            xt_ap[:, 0, j0 * P : (j0 + 4) * P],
                            start=True,
                            stop=False,
                        )
                        nc.tensor.matmul(
                            hps[:],
                            w1b_sb[:],
                            xt_ap[:, 1, j0 * P : (j0 + 4) * P],
                            start=False,
                            stop=True,
                        )
                        hsb = hsbpool.tile([P, 4 * P], BF16)
                        nc.scalar.activation(
                            hsb[:], hps[:],
                            mybir.ActivationFunctionType.Tanh,
                            bias=b1_sb[:],
                            scale=1.0 / W1SCALE,
                        )
                        hsbs.append(hsb)
                    return hsbs

                def ind(g8):
                    c0 = g8 * 8
                    ind8 = ohpool.tile([P, 8 * BLKG], BF16, tag="ind8")
                    io_v = io_sb[:].rearrange("p (j g) -> p j g", j=8)
                    blr_v = blr_sb[:, c0 : c0 + 8].unsqueeze(-1)                        .broadcast_to([P, 8, BLKG])
                    ind8_v = ind8[:].rearrange("p (j g) -> p j g", j=8)
                    nc.vector.tensor_tensor(
                        ind8_v, io_v, blr_v, mybir.AluOpType.is_equal
                    )
                    return ind8_v

                def logits_oh(g8, xb, hsbs, ind8_v):
                    lg = lgpool.tile([P, 8], F32)
                    oh8a = ohpool.tile([P, 4 * BLKG], BF16, tag="oh8a")
                    oh8b = ohpool.tile([P, 4 * BLKG], BF16, tag="oh8b")
                    for h in range(2):
                        for j in range(4 * h, 4 * h + 4):
                            nc.tensor.matmul(
                                lg[:, j : j + 1],
                                hsbs[j // 4][
                                    :, (j % 4) * P : (j % 4 + 1) * P
                                ],
                                w2_sb[:],
                                start=True,
                                stop=True,
                            )
                        e4 = epool.tile([P, 4], BF16, tag=f"e4{h}")
                        nc.scalar.activation(
                            e4[:], lg[:, 4 * h : 4 * h + 4],
                            mybir.ActivationFunctionType.Exp,
                        )
                        ohh = (oh8a, oh8b)[h]
                        e4_v = e4[:].unsqueeze(-1).broadcast_to(
                            [P, 4, BLKG]
                        )
                        ohh_v = ohh[:].rearrange("p (j g) -> p j g", j=4)
                        nc.vector.tensor_tensor(
                            ohh_v, ind8_v[:, 4 * h : 4 * h + 4, :], e4_v,
                            mybir.AluOpType.mult,
                        )
                    return oh8a, oh8b

                # phase 1: MLP for g0 (xt stream)
                g8_0, xb_0, xt_0 = gdat[0]
                hsbs_0 = mlp(g8_0, xt_0)
                iv_0 = ind(g8_0)
                # phase 2: pooled burst for the two groups two iters back
                # (xs stream, 16 chunks) - covers tanh(g0)
                if len(prevq) >= 2:
                    for _ in range(2):
                        poa, pob, pxb, pc0 = prevq.pop(0)
                        for j in range(8):
                            po = (poa, pob)[j // 4]
                            flush_one(
                                (po[:, (j % 4) * BLKG :
                                     (j % 4 + 1) * BLKG],
                                 pxb, j, pc0 + j)
                            )
                # phase 3: MLP for g1 (xt stream)
                g8_1, xb_1, xt_1 = gdat[1]
                hsbs_1 = mlp(g8_1, xt_1)
                iv_1 = ind(g8_1)
                # phase 4: logits + oh for both groups (w2 stream)
                oh_0 = logits_oh(g8_0, xb_0, hsbs_0, iv_0)
                oh_1 = logits_oh(g8_1, xb_1, hsbs_1, iv_1)
                prevq.append((oh_0[0], oh_0[1], xb_0, g8_0 * 8))
                prevq.append((oh_1[0], oh_1[1], xb_1, g8_1 * 8))
            for poa, pob, pxb, pc0 in prevq:
                for j in range(8):
                    po = (poa, pob)[j // 4]
                    flush_one(
                        (po[:, (j % 4) * BLKG : (j % 4 + 1) * BLKG],
                         pxb, j, pc0 + j)
                    )

            # ---- epilogue: out[g] = pooled[g] / max(denom[g], tiny)
            recs = []
            for b in range(NBLK):
                acc = pooled_out(b)
                dmax = outpool.tile([BLKG, 1], F32, tag=f"dmax{b}", name=f"dmax{b}")
                rec = outpool.tile([BLKG, 1], F32, tag=f"rec{b}", name=f"rec{b}")
                nc.vector.tensor_scalar_max(dmax[:], acc[:, D : D + 1], 1e-30)
                nc.vector.reciprocal(rec[:], dmax[:])
                recs.append(rec)
            for b in range(NBLK):
                osb = outpool.tile([BLKG, D], F32, tag="osb")
                nc.scalar.mul(osb[:], pooled_out(b)[:, 0:D], recs[b][:])
                nc.sync.dma_start(out[b * BLKG : (b + 1) * BLKG, :], osb[:])

    nc.compile()
    return nc


# ----------------------------------------------------------------- kernel()

def kernel(**inputs):
    global LAST_RESULT
    x = np.asarray(inputs["x"])
    batch = np.asarray(inputs["batch"])
    W1 = np.asarray(inputs["W1"])
    b1 = np.asarray(inputs["b1"])
    W2 = np.asarray(inputs["W2"])
    # b2 cancels in the softmax; unused.

    seg, cpbs, starts, ch = make_plan(batch)
    in_maps = build_inputs(x, batch, W1, b1, W2, seg, starts, ch)
    nc = build_bass(ch, starts)
    res = bass_utils.run_bass_kernel_spmd(
        nc, in_maps, list(range(NCORES))
    )
    LAST_RESULT = res
    out = np.concatenate(
        [np.asarray(res.results[c]["out"]) for c in range(NCORES)], axis=0
    )
    return out.astype(np.float32)
